# revision 13
# baseline (speedup 1.0000x reference)
"""Trainium2 Bass kernel for MimiAttention (GQA + RoPE + causal softmax).

Problem: B=2, S=2048, H=1024, NH=16 q-heads, NKV=4 kv-heads, HD=64.
Sharding: 8 cores = 2 (batch) x 4 (kv-group).  Each core computes one batch's
attention for one GQA group (4 q-heads sharing 1 kv head) and the partial
o-projection for those heads; the host sums the 4 partials per batch.

v4 design notes (all bf16, fp32 psum):
  * RoPE hat-trick as in the baseline: wqk columns carry [q; q2] per head
    (q2 = sign-permuted rows), qhat = proj * cs, khat = [k_rot; k_rot] where
    k_rot = (k*cos)+(k2*sin) folded with DVE adds (no PE fold matmul).
  * Scores are computed transposed (scoresT[j, i]) per key-tile row into two
    ping-pong PSUM "feed" regions of [128, 1024]; ONE exp activation per
    segment (<=1024 cols) drastically cuts ACT fixed cost vs per-512-chunk.
  * Causal diagonal handled by an in-place DVE multiply with an upper-tri
    mask on the exp output (cheap 2x-mode [128,128] op).
  * attnV accumulates out[i, d|den] in three persistent PSUM banks
    (65-wide slices, column 64 = softmax denominator via ones-column in v).
  * attn[i,c] -> aT[c,i] transposes done by DMA-transpose (xbar), not PE.
  * o-projection streams during head 3 through the 1-bank work psum and
    finishes after attention using the freed feed banks.
"""

import numpy as np
import ml_dtypes

B, S, H = 2, 2048, 1024
NH, NKV, HD = 16, 4, 64
G = NH // NKV            # 4 q-heads per kv head
THETA = 10000.0
N_CORES = 8

BF16 = ml_dtypes.bfloat16

NSB = S // 512           # 4 chunks of 512
NST = S // 128           # 16 tiles of 128
KC = H // 128            # 8 contraction chunks
SCALE = float(1.0 / np.sqrt(HD))


def _build_nc():
    import concourse.mybir as mybir
    import concourse.tile as tile
    from concourse.tile import add_dep_helper
    from concourse import bacc

    f32 = mybir.dt.float32
    bf16 = mybir.dt.bfloat16

    nc = bacc.Bacc("TRN2", target_bir_lowering=False)

    xTd = nc.dram_tensor("xT", [H, S], bf16, kind="ExternalInput")
    wqkd = nc.dram_tensor("wqkT", [H, 640], bf16, kind="ExternalInput")
    wvd = nc.dram_tensor("wvT", [H, HD], bf16, kind="ExternalInput")
    csd = nc.dram_tensor("cs", [128, S], bf16, kind="ExternalInput")
    wod = nc.dram_tensor("woT", [G * HD, H], bf16, kind="ExternalInput")
    trid = nc.dram_tensor("trimask", [128, 128], bf16, kind="ExternalInput")
    djd = nc.dram_tensor("dupJ", [128, 128], bf16, kind="ExternalInput")
    oTd = nc.dram_tensor("oT", [H, S], bf16, kind="ExternalOutput")

    with tile.TileContext(nc) as tc:
        import contextlib
        ctx = contextlib.ExitStack()
        with ctx:
            consts = ctx.enter_context(tc.tile_pool(name="consts", bufs=1))
            acts = ctx.enter_context(tc.tile_pool(name="acts", bufs=1))
            ep = ctx.enter_context(tc.tile_pool(name="exps", bufs=3))
            rcp = ctx.enter_context(tc.tile_pool(name="rcp", bufs=6))
            otp = ctx.enter_context(tc.tile_pool(name="ot", bufs=8))
            # PSUM: 3 banks attnV accum + 2+2 banks exp feed + 1 bank work
            pav = ctx.enter_context(
                tc.tile_pool(name="ps_av", bufs=1, space="PSUM"))
            pfa = ctx.enter_context(
                tc.tile_pool(name="ps_fa", bufs=1, space="PSUM"))
            pfb = ctx.enter_context(
                tc.tile_pool(name="ps_fb", bufs=1, space="PSUM"))
            pw = ctx.enter_context(
                tc.tile_pool(name="ps_w", bufs=1, space="PSUM"))

            # ---- input DMAs: first 512 cols of xt (per kc) + weights first
            # so the prologue projections can start early.
            xt_sb = consts.tile([128, KC, S], bf16, tag="xt")
            wqk_sb = consts.tile([128, KC, 640], bf16, tag="wqk")
            for kc in range(KC):
                nc.sync.dma_start(xt_sb[:, kc, 0:512],
                                  xTd[kc * 128:(kc + 1) * 128, 0:512])
                nc.sync.dma_start(
                    wqk_sb[:, kc, :], wqkd[kc * 128:(kc + 1) * 128, :])
            cs_sb = consts.tile([128, S], bf16, tag="cs")
            nc.sync.dma_start(cs_sb, csd[:, :])
            tri_sb = consts.tile([128, 128], bf16, tag="tri")
            nc.sync.dma_start(tri_sb, trid[:, :])
            dj_sb = consts.tile([128, 128], bf16, tag="dj")
            nc.sync.dma_start(dj_sb, djd[:, :])
            wv_sb = consts.tile([128, KC, HD], bf16, tag="wv")
            nc.sync.dma_start(wv_sb, wvd.rearrange("(kc p) m -> p kc m", p=128))
            for kc in range(KC):
                nc.sync.dma_start(xt_sb[:, kc, 512:S],
                                  xTd[kc * 128:(kc + 1) * 128, 512:S])
            wo_sb = consts.tile([128, 2, H], bf16, tag="wo")
            nc.sync.dma_start(wo_sb, wod.rearrange("(kc p) m -> p kc m", p=128))

            qhat = [acts.tile([128, S], bf16, tag=f"qh{m}", name=f"qhat{m}")
                    for m in range(G)]
            khat = acts.tile([128, S], bf16, tag="khat")
            ktmp = acts.tile([128, S], bf16, tag="ktmp")
            v_sb = acts.tile([128, NST, HD + 1], bf16, tag="vsb")
            attn_n = acts.tile([128, NST, G * HD], bf16, tag="attn")
            aT = acts.tile([128, 2, S], bf16, tag="aT")

            # attnV accumulators: slice `it` = bank[it//7][:, (it%7)*65 :+65]
            avb = [pav.tile([128, w], f32, tag=f"av{b}", name=f"avb{b}")
                   for b, w in ((0, 455), (1, 455), (2, 130))]

            def av_slice(it):
                b, o = it // 7, (it % 7) * 65
                return avb[b][:, o:o + 65]

            feedA = pfa.tile([128, 1024], f32, tag="fA")
            feedB = pfb.tile([128, 1024], f32, tag="fB")
            feeds = [feedA, feedB]

            def proj_psum(m, n, ps):
                """8 contraction matmuls for output group m, col chunk n."""
                col = n * 512
                for kc in range(KC):
                    nc.tensor.matmul(
                        ps, wqk_sb[:, kc, m * 128:(m + 1) * 128],
                        xt_sb[:, kc, col:col + 512],
                        start=(kc == 0), stop=(kc == KC - 1))

            def drain_q(h, n, ps):
                col = n * 512
                nc.vector.tensor_mul(
                    qhat[h][:, col:col + 512], ps, cs_sb[:, col:col + 512])

            def drain_k(n, ps, psf):
                # ktmp = proj * cs, then fold matmul J @ ktmp -> khat chunk
                # (J duplicates and sums the two 64-row halves; partition
                # crossing must go through the PE).
                col = n * 512
                nc.vector.tensor_mul(
                    ktmp[:, col:col + 512], ps, cs_sb[:, col:col + 512])
                nc.tensor.matmul(psf, dj_sb, ktmp[:, col:col + 512],
                                 start=True, stop=True)
                nc.vector.tensor_copy(khat[:, col:col + 512], psf)

            def v_proj(st):
                psv = pw.tile([128, HD], f32, tag="w", name="psv")
                for kc in range(KC):
                    nc.tensor.matmul(
                        psv, xt_sb[:, kc, st * 128:(st + 1) * 128],
                        wv_sb[:, kc, :],
                        start=(kc == 0), stop=(kc == KC - 1))
                nc.vector.tensor_copy(v_sb[:, st, 0:HD], psv)

            # ---- prologue: k-path and q0 interleaved through 3 psum slots
            nc.gpsimd.memset(v_sb[:, :, HD:HD + 1], 1.0)
            for n in range(NSB):
                psk = (pfa.tile([128, 512], f32, tag="fA", name="psk")
                       if n % 2 == 0 else
                       pfb.tile([128, 512], f32, tag="fB", name="psk"))
                proj_psum(G, n, psk)
                psf = pw.tile([128, 512], f32, tag="w", name="psf")
                drain_k(n, psk, psf)
                psq = (pfa.tile([128, 512], f32, tag="fA", name="psq")
                       if n % 2 == 1 else
                       pfb.tile([128, 512], f32, tag="fB", name="psq"))
                proj_psum(0, n, psq)
                drain_q(0, n, psq)
            v_proj(0)

            # oproj state -------------------------------------------------
            oproj_pending = []   # list of (g, hc)

            def oproj_chunk(g, hc, ps, drain_eng):
                col = g * 512
                for kc2 in range(2):
                    nc.tensor.matmul(
                        ps, wo_sb[:, kc2, hc * 128:(hc + 1) * 128],
                        aT[:, kc2, col:col + 512],
                        start=(kc2 == 0), stop=(kc2 == 1))
                ot = otp.tile([128, 512], bf16, tag="ot", name="ot")
                if drain_eng == 0:
                    nc.vector.tensor_copy(ot, ps)
                else:
                    nc.scalar.copy(ot, ps)
                nc.sync.dma_start(
                    oTd[hc * 128:(hc + 1) * 128, col:col + 512], ot)

            # ---- attention ----------------------------------------------
            seg_idx = 0          # global ping-pong counter for feed regions
            for h in range(G):
                bank_first = {}
                for jt in range(NST):
                    lo = jt * 128
                    cols = S - lo
                    et = ep.tile([128, S], bf16, tag="e", name=f"e{h}_{jt}")
                    lhsT = khat[:, lo:lo + 128]
                    segs = [(lo, min(1024, cols))]
                    if cols > 1024:
                        segs.append((lo + 1024, cols - 1024))
                    for (off, ln) in segs:
                        region = feeds[seg_idx % 2]
                        seg_idx += 1
                        # scores matmuls in 512-aligned chunks of the region
                        done = 0
                        while done < ln:
                            cl = min(512, ln - done)
                            nc.tensor.matmul(
                                region[:, done:done + cl], lhsT,
                                qhat[h][:, off + done:off + done + cl],
                                start=True, stop=True)
                            done += cl
                        nc.scalar.activation(
                            et[:, off:off + ln], region[:, 0:ln],
                            mybir.ActivationFunctionType.Exp, scale=SCALE)
                    # causal mask on the diagonal tile (in place, Pool —
                    # the only sbuf-only elementwise work we can offload)
                    nc.gpsimd.tensor_mul(et[:, lo:lo + 128],
                                         et[:, lo:lo + 128], tri_sb)

                    # attnV: batch 1 = tiles it in [jt, jt+7] (descending,
                    # masked diagonal last), batch 2 = the rest.
                    b1_hi = min(jt + 7, NST - 1)
                    order = list(range(b1_hi, jt - 1, -1)) + \
                        list(range(NST - 1, b1_hi, -1))
                    for it in order:
                        b = it // 7
                        first = jt == 0 and b not in bank_first
                        mm = nc.tensor.matmul(
                            av_slice(it), et[:, it * 128:(it + 1) * 128],
                            v_sb[:, jt, :],
                            start=first, stop=(it == jt),
                            skip_group_check=True)
                        if first:
                            bank_first[b] = mm
                        elif jt == 0:
                            add_dep_helper(mm.ins, bank_first[b].ins,
                                           sync=False,
                                           reason="bank clear first")

                    # slice it=jt is complete: normalize (recip on DVE,
                    # scale on Pool) into attn_n
                    pso = av_slice(jt)
                    rc = rcp.tile([128, 1], f32, tag="rc", name="rc")
                    nc.vector.reciprocal(rc, pso[:, HD:HD + 1])
                    nc.vector.tensor_scalar_mul(
                        attn_n[:, jt, h * HD:(h + 1) * HD], pso[:, 0:HD], rc)

                    # head-pair transpose (after odd head writes its slice)
                    if h % 2 == 1:
                        hp = h // 2
                        nc.sync.dma_start_transpose(
                            aT[:, hp, jt * 128:(jt + 1) * 128],
                            attn_n[:, jt, hp * 128:(hp + 1) * 128])

                    # interleaves
                    if h == 0 and jt < NST - 1:
                        v_proj(jt + 1)
                    if h < G - 1 and jt in (1, 4, 7, 10):
                        n = (jt - 1) // 3
                        psq = pw.tile([128, 512], f32, tag="w", name="psq2")
                        proj_psum(h + 1, n, psq)
                        drain_q(h + 1, n, psq)
                    if h == G - 1:
                        if jt >= 3 and jt % 4 == 3:
                            g = jt // 4
                            oproj_pending.extend((g, hc) for hc in range(KC))
                        # stream up to 2 chunks per row through the w bank
                        for _ in range(2):
                            if oproj_pending and jt >= 4:
                                g, hc = oproj_pending.pop(0)
                                ps = pw.tile([128, 512], f32, tag="w",
                                             name="pso2")
                                oproj_chunk(g, hc, ps, drain_eng=0)

            # ---- tail: remaining oproj chunks through all free psum ----
            ti = 0
            while oproj_pending:
                g, hc = oproj_pending.pop(0)
                which = ti % 3
                if which == 0:
                    ps = pw.tile([128, 512], f32, tag="w", name="psot")
                elif which == 1:
                    ps = pfa.tile([128, 512], f32, tag="fA", name="psot")
                else:
                    ps = pfb.tile([128, 512], f32, tag="fB", name="psot")
                oproj_chunk(g, hc, ps, drain_eng=ti % 2)
                ti += 1

    nc.finalize()
    return nc


def _host_inputs(hidden_states, position_ids, wq, wk, wv, wo):
    """Build the 8 per-core input maps."""
    def w2_of(w):
        # w: [64, H] rows of one head; returns sign-permuted rows
        w2 = np.empty_like(w)
        w2[:32] = -w[32:64]
        w2[32:] = w[:32]
        return w2

    trimask = np.triu(np.ones((128, 128), np.float32)).astype(BF16)
    dupJ = np.zeros((128, 128), np.float32)
    for p in range(128):
        dupJ[p, p % 64] = 1.0
        dupJ[p, p % 64 + 64] = 1.0
    dupJ = dupJ.astype(BF16)

    in_maps = []
    for core in range(N_CORES):
        b, kv = core // NKV, core % NKV
        xT = np.ascontiguousarray(hidden_states[b].T).astype(BF16)

        cols = []
        for i in range(G):
            h = kv * G + i
            wqh = wq[h * HD:(h + 1) * HD]
            cols.append(wqh.T)
            cols.append(w2_of(wqh).T)
        wkh = wk[kv * HD:(kv + 1) * HD]
        cols.append(wkh.T)
        cols.append(w2_of(wkh).T)
        wqkT = np.ascontiguousarray(np.concatenate(cols, axis=1)).astype(BF16)

        wvT = np.ascontiguousarray(wv[kv * HD:(kv + 1) * HD].T).astype(BF16)
        woT = np.ascontiguousarray(
            wo[:, kv * G * HD:(kv + 1) * G * HD].T).astype(BF16)

        inv = 1.0 / (THETA ** (np.arange(0, HD, 2, dtype=np.float32) / HD))
        freqs = position_ids[b].astype(np.float32)[:, None] * inv[None, :]
        emb = np.concatenate([freqs, freqs], axis=-1)       # [S, 64]
        cs = np.concatenate([np.cos(emb).T, np.sin(emb).T], axis=0)  # [128, S]
        cs = np.ascontiguousarray(cs).astype(BF16)

        in_maps.append({
            "xT": xT, "wqkT": wqkT, "wvT": wvT, "cs": cs, "woT": woT,
            "trimask": trimask, "dupJ": dupJ,
        })
    return in_maps


_NC_CACHE = {}


def run_cores(in_maps, trace=False, trace_kwargs=None):
    from concourse.bass_utils import run_bass_kernel_spmd
    if "nc" not in _NC_CACHE:
        _NC_CACHE["nc"] = _build_nc()
    nc = _NC_CACHE["nc"]
    return run_bass_kernel_spmd(
        nc, in_maps, core_ids=list(range(N_CORES)),
        trace=trace, **(trace_kwargs or {}))


def kernel(hidden_states, attention_mask, position_ids, wq, wk, wv, wo):
    hidden_states = np.asarray(hidden_states, dtype=np.float32)
    position_ids = np.asarray(position_ids)
    wq = np.asarray(wq, dtype=np.float32)
    wk = np.asarray(wk, dtype=np.float32)
    wv = np.asarray(wv, dtype=np.float32)
    wo = np.asarray(wo, dtype=np.float32)

    in_maps = _host_inputs(hidden_states, position_ids, wq, wk, wv, wo)
    res = run_cores(in_maps)

    out = np.zeros((B, S, H), np.float32)
    for core in range(N_CORES):
        b = core // NKV
        out[b] += res.results[core]["oT"].T.astype(np.float32)
    return out


# revision 16
# speedup vs baseline: 1.2002x; 1.2002x over previous
"""Trainium2 Bass kernel for MimiAttention (GQA + RoPE + causal softmax).

Problem: B=2, S=2048, H=1024, NH=16 q-heads, NKV=4 kv-heads, HD=64.
Sharding: 8 cores = 2 (batch) x 4 (kv-group).  Each core computes one batch's
attention for one GQA group (4 q-heads sharing 1 kv head) and the partial
o-projection for those heads; the host sums the 4 partials per batch.

v5 design (all bf16 matmuls, fp32 psum):
  * RoPE hat-trick (as baseline): wqk columns carry [q; q2] per head, qhat =
    proj * cs; khat = [k_rot; k_rot] via the J-fold matmul.
  * Scores transposed (scoresT[j, i]) per key-tile row, streamed through two
    ping-pong [128, 1024] PSUM feed regions; ONE exp per segment (<= 2 per
    row) minimizes the ACT fixed cost.  Causal diag masked in place on Pool.
  * Software pipeline: scores+exp for row r are issued BEFORE attnV of row
    r-1 so the PE never stalls behind the Pool mask / exp chain.
  * attnV accumulates [i, v|den] slices in 3 persistent psum banks
    (65-wide slices; col 64 = denominator via the ones column of v).
  * attn[i,c] -> aT[c,i] via PE transposes through the 1-bank work slot
    (pair 0 during head 2, pair 1 inline during head 3).
  * o-projection: 2 chunks per row during head 3 through the work bank,
    remainder after attention through 3 rotating psum slots; output DMAs
    batched 4 chunks each.
"""

import numpy as np
import ml_dtypes

B, S, H = 2, 2048, 1024
NH, NKV, HD = 16, 4, 64
G = NH // NKV            # 4 q-heads per kv head
THETA = 10000.0
N_CORES = 8

BF16 = ml_dtypes.bfloat16

NSB = S // 512           # 4 chunks of 512
NST = S // 128           # 16 tiles of 128
KC = H // 128            # 8 contraction chunks
SCALE = float(1.0 / np.sqrt(HD))


def _build_nc():
    import concourse.mybir as mybir
    import concourse.tile as tile
    from concourse.tile import add_dep_helper
    from concourse import bacc

    f32 = mybir.dt.float32
    bf16 = mybir.dt.bfloat16

    nc = bacc.Bacc("TRN2", target_bir_lowering=False)

    xTd = nc.dram_tensor("xT", [H, S], bf16, kind="ExternalInput")
    wqkd = nc.dram_tensor("wqkT", [H, 640], bf16, kind="ExternalInput")
    wvd = nc.dram_tensor("wvT", [H, HD], bf16, kind="ExternalInput")
    csd = nc.dram_tensor("cs", [128, S], bf16, kind="ExternalInput")
    wod = nc.dram_tensor("woT", [G * HD, H], bf16, kind="ExternalInput")
    trid = nc.dram_tensor("trimask", [128, 128], bf16, kind="ExternalInput")
    djd = nc.dram_tensor("dupJ", [128, 128], bf16, kind="ExternalInput")
    idd = nc.dram_tensor("ident", [128, 128], bf16, kind="ExternalInput")
    oTd = nc.dram_tensor("oT", [H, S], bf16, kind="ExternalOutput")

    with tile.TileContext(nc) as tc:
        import contextlib
        ctx = contextlib.ExitStack()
        with ctx:
            consts = ctx.enter_context(tc.tile_pool(name="consts", bufs=1))
            acts = ctx.enter_context(tc.tile_pool(name="acts", bufs=1))
            ep = ctx.enter_context(tc.tile_pool(name="exps", bufs=3))
            rcp = ctx.enter_context(tc.tile_pool(name="rcp", bufs=6))
            otp = ctx.enter_context(tc.tile_pool(name="ot", bufs=3))
            pav = ctx.enter_context(
                tc.tile_pool(name="ps_av", bufs=1, space="PSUM"))
            pfa = ctx.enter_context(
                tc.tile_pool(name="ps_fa", bufs=1, space="PSUM"))
            pfb = ctx.enter_context(
                tc.tile_pool(name="ps_fb", bufs=1, space="PSUM"))
            pw = ctx.enter_context(
                tc.tile_pool(name="ps_w", bufs=1, space="PSUM"))

            # ---- input DMAs: first 512 cols of xt (per kc) + weights first
            xt_sb = consts.tile([128, KC, S], bf16, tag="xt")
            wqk_sb = consts.tile([128, KC, 640], bf16, tag="wqk")
            for kc in range(KC):
                nc.sync.dma_start(xt_sb[:, kc, 0:512],
                                  xTd[kc * 128:(kc + 1) * 128, 0:512])
                nc.sync.dma_start(
                    wqk_sb[:, kc, :], wqkd[kc * 128:(kc + 1) * 128, :])
            cs_sb = consts.tile([128, S], bf16, tag="cs")
            nc.sync.dma_start(cs_sb, csd[:, :])
            tri_sb = consts.tile([128, 128], bf16, tag="tri")
            nc.sync.dma_start(tri_sb, trid[:, :])
            dj_sb = consts.tile([128, 128], bf16, tag="dj")
            nc.sync.dma_start(dj_sb, djd[:, :])
            id_sb = consts.tile([128, 128], bf16, tag="id")
            nc.sync.dma_start(id_sb, idd[:, :])
            wv_sb = consts.tile([128, KC, HD], bf16, tag="wv")
            nc.sync.dma_start(wv_sb, wvd.rearrange("(kc p) m -> p kc m", p=128))
            for kc in range(KC):
                nc.sync.dma_start(xt_sb[:, kc, 512:S],
                                  xTd[kc * 128:(kc + 1) * 128, 512:S])
            wo_sb = consts.tile([128, 2, H], bf16, tag="wo")
            nc.sync.dma_start(wo_sb, wod.rearrange("(kc p) m -> p kc m", p=128))

            qhat = [acts.tile([128, S], bf16, tag=f"qh{m}", name=f"qhat{m}")
                    for m in range(G)]
            khat = acts.tile([128, S], bf16, tag="khat")
            ktmp = acts.tile([128, S], bf16, tag="ktmp")
            v_sb = acts.tile([128, NST, HD + 1], bf16, tag="vsb")
            attn_n = acts.tile([128, NST, G * HD], bf16, tag="attn")
            aT = acts.tile([128, 2, S], bf16, tag="aT")

            avb = [pav.tile([128, w], f32, tag=f"av{b}", name=f"avb{b}")
                   for b, w in ((0, 455), (1, 455), (2, 130))]

            def av_slice(it):
                b, o = it // 7, (it % 7) * 65
                return avb[b][:, o:o + 65]

            feedA = pfa.tile([128, 1024], f32, tag="fA")
            feedB = pfb.tile([128, 1024], f32, tag="fB")
            feeds = [feedA, feedB]

            def proj_psum(m, n, ps):
                col = n * 512
                for kc in range(KC):
                    nc.tensor.matmul(
                        ps, wqk_sb[:, kc, m * 128:(m + 1) * 128],
                        xt_sb[:, kc, col:col + 512],
                        start=(kc == 0), stop=(kc == KC - 1))

            def q_chunk(h, n):
                ps = pw.tile([128, 512], f32, tag="w", name="psq")
                proj_psum(h, n, ps)
                col = n * 512
                nc.vector.tensor_mul(
                    qhat[h][:, col:col + 512], ps, cs_sb[:, col:col + 512])

            def k_chunk(n):
                ps = pw.tile([128, 512], f32, tag="w", name="psk")
                proj_psum(G, n, ps)
                col = n * 512
                nc.vector.tensor_mul(
                    ktmp[:, col:col + 512], ps, cs_sb[:, col:col + 512])
                psf = pw.tile([128, 512], f32, tag="w", name="psf")
                nc.tensor.matmul(psf, dj_sb, ktmp[:, col:col + 512],
                                 start=True, stop=True)
                # fold drain on ACT: it is idle during the prologue
                nc.scalar.copy(khat[:, col:col + 512], psf)

            def v_proj(st):
                psv = pw.tile([128, HD], f32, tag="w", name="psv")
                for kc in range(KC):
                    nc.tensor.matmul(
                        psv, xt_sb[:, kc, st * 128:(st + 1) * 128],
                        wv_sb[:, kc, :],
                        start=(kc == 0), stop=(kc == KC - 1))
                nc.vector.tensor_copy(v_sb[:, st, 0:HD], psv)

            def transpose_slice(hp, it):
                psx = pw.tile([128, 128], bf16, tag="w", name="pst")
                nc.tensor.transpose(
                    psx, attn_n[:, it, hp * 128:(hp + 1) * 128], id_sb)
                nc.vector.tensor_copy(aT[:, hp, it * 128:(it + 1) * 128], psx)

            # ---- prologue
            nc.gpsimd.memset(v_sb[:, :, HD:HD + 1], 1.0)
            k_chunk(0)
            q_chunk(0, 0)
            q_chunk(0, 1)

            seg_counter = [0]

            def scores_row(h, jt, et):
                lo = jt * 128
                cols = S - lo
                lhsT = khat[:, lo:lo + 128]
                segs = [(lo, min(1024, cols))]
                if cols > 1024:
                    segs.append((lo + 1024, cols - 1024))
                for si, (off, ln) in enumerate(segs):
                    region = feeds[seg_counter[0] % 2]
                    seg_counter[0] += 1
                    done = 0
                    while done < ln:
                        cl = min(512, ln - done)
                        nc.tensor.matmul(
                            region[:, done:done + cl], lhsT,
                            qhat[h][:, off + done:off + done + cl],
                            start=True, stop=True)
                        done += cl
                    nc.scalar.activation(
                        et[:, off:off + ln], region[:, 0:ln],
                        mybir.ActivationFunctionType.Exp, scale=SCALE)
                    if si == 0 and h == 0 and jt == 0:
                        # earliest point where q0 chunks 2,3 are needed
                        q_chunk(0, 2)
                        q_chunk(0, 3)
                # causal mask on diag tile: in place on Pool (hidden behind
                # the next row's scores matmuls by the software pipeline)
                nc.gpsimd.tensor_mul(et[:, lo:lo + 128],
                                     et[:, lo:lo + 128], tri_sb)

            attnv_state = {}   # h -> bank_first dict

            def attnv_row(h, jt, et):
                bank_first = attnv_state.setdefault(h, {})
                lo = jt * 128
                b1_hi = min(jt + 7, NST - 1)
                order = list(range(b1_hi, jt - 1, -1)) + \
                    list(range(NST - 1, b1_hi, -1))
                for it in order:
                    b = it // 7
                    first = jt == 0 and b not in bank_first
                    mm = nc.tensor.matmul(
                        av_slice(it), et[:, it * 128:(it + 1) * 128],
                        v_sb[:, jt, :],
                        start=first, stop=(it == jt),
                        skip_group_check=True)
                    if first:
                        bank_first[b] = mm
                    elif jt == 0:
                        add_dep_helper(mm.ins, bank_first[b].ins,
                                       sync=False,
                                       reason="bank clear first")
                pso = av_slice(jt)
                rc = rcp.tile([128, 1], f32, tag="rc", name="rc")
                nc.vector.reciprocal(rc, pso[:, HD:HD + 1])
                nc.vector.tensor_scalar_mul(
                    attn_n[:, jt, h * HD:(h + 1) * HD], pso[:, 0:HD], rc)

            # oproj -------------------------------------------------------
            oproj_pending = [(g, hc) for g in range(NSB) for hc in range(KC)]
            ot_state = {}  # g -> current ot_big tile

            def oproj_chunk(ps, drain_eng):
                g, hc = oproj_pending.pop(0)
                col = g * 512
                for kc2 in range(2):
                    nc.tensor.matmul(
                        ps, wo_sb[:, kc2, hc * 128:(hc + 1) * 128],
                        aT[:, kc2, col:col + 512],
                        start=(kc2 == 0), stop=(kc2 == 1))
                if hc % 4 == 0:
                    ot_state[g] = otp.tile([128, 4, 512], bf16, tag="otb",
                                           name="otb")
                ot = ot_state[g]
                if drain_eng == 0:
                    nc.vector.tensor_copy(ot[:, hc % 4, :], ps)
                else:
                    nc.scalar.copy(ot[:, hc % 4, :], ps)
                if hc % 4 == 3:
                    r0 = (hc // 4) * 512
                    nc.sync.dma_start(
                        oTd[r0:r0 + 512, col:col + 512].rearrange(
                            "(c p) m -> p c m", p=128), ot)

            # ---- main pipelined loop ------------------------------------
            seq = [(h, jt) for h in range(G) for jt in range(NST)]
            prev = None
            for (h, jt) in seq:
                et = ep.tile([128, S], bf16, tag="e", name=f"e{h}_{jt}")
                scores_row(h, jt, et)
                if prev is not None:
                    attnv_row(*prev)
                ph, pjt = (prev[0], prev[1]) if prev else (None, None)
                prev = (h, jt, et)

                # interleaved producer work (kept off the row critical path)
                if h == 0:
                    if jt == 0:
                        v_proj(0)
                    if jt < NST - 1:
                        v_proj(jt + 1)
                    if jt in (2, 6, 10):
                        k_chunk(jt // 4 + 1)
                if h < G - 1 and jt in (1, 4, 7, 10):
                    q_chunk(h + 1, (jt - 1) // 3)
                if h == 2:
                    # pair-0 transposes: slice (1, it) fully normed by now
                    transpose_slice(0, jt)
                if h == 3:
                    if pjt is not None and ph == 3:
                        transpose_slice(1, pjt)
                    if jt >= 5:
                        for _ in range(2):
                            if oproj_pending and \
                                    oproj_pending[0][0] * 4 + 4 <= jt:
                                ps = pw.tile([128, 512], f32, tag="w",
                                             name="psow")
                                oproj_chunk(ps, drain_eng=0)

            # flush: last attnV row + transpose + remaining oproj
            attnv_row(*prev)
            transpose_slice(1, NST - 1)
            ti = 0
            while oproj_pending:
                which = ti % 3
                if which == 0:
                    ps = pw.tile([128, 512], f32, tag="w", name="psot")
                elif which == 1:
                    ps = pfa.tile([128, 512], f32, tag="fA", name="psot")
                else:
                    ps = pfb.tile([128, 512], f32, tag="fB", name="psot")
                oproj_chunk(ps, drain_eng=ti % 2)
                ti += 1

    nc.finalize()
    return nc


def _host_inputs(hidden_states, position_ids, wq, wk, wv, wo):
    """Build the 8 per-core input maps."""
    def w2_of(w):
        # w: [64, H] rows of one head; returns sign-permuted rows
        w2 = np.empty_like(w)
        w2[:32] = -w[32:64]
        w2[32:] = w[:32]
        return w2

    trimask = np.triu(np.ones((128, 128), np.float32)).astype(BF16)
    dupJ = np.zeros((128, 128), np.float32)
    for p in range(128):
        dupJ[p, p % 64] = 1.0
        dupJ[p, p % 64 + 64] = 1.0
    dupJ = dupJ.astype(BF16)
    ident = np.eye(128, dtype=np.float32).astype(BF16)

    in_maps = []
    for core in range(N_CORES):
        b, kv = core // NKV, core % NKV
        xT = np.ascontiguousarray(hidden_states[b].T).astype(BF16)

        cols = []
        for i in range(G):
            h = kv * G + i
            wqh = wq[h * HD:(h + 1) * HD]
            cols.append(wqh.T)
            cols.append(w2_of(wqh).T)
        wkh = wk[kv * HD:(kv + 1) * HD]
        cols.append(wkh.T)
        cols.append(w2_of(wkh).T)
        wqkT = np.ascontiguousarray(np.concatenate(cols, axis=1)).astype(BF16)

        wvT = np.ascontiguousarray(wv[kv * HD:(kv + 1) * HD].T).astype(BF16)
        woT = np.ascontiguousarray(
            wo[:, kv * G * HD:(kv + 1) * G * HD].T).astype(BF16)

        inv = 1.0 / (THETA ** (np.arange(0, HD, 2, dtype=np.float32) / HD))
        freqs = position_ids[b].astype(np.float32)[:, None] * inv[None, :]
        emb = np.concatenate([freqs, freqs], axis=-1)       # [S, 64]
        cs = np.concatenate([np.cos(emb).T, np.sin(emb).T], axis=0)  # [128, S]
        cs = np.ascontiguousarray(cs).astype(BF16)

        in_maps.append({
            "xT": xT, "wqkT": wqkT, "wvT": wvT, "cs": cs, "woT": woT,
            "trimask": trimask, "dupJ": dupJ, "ident": ident,
        })
    return in_maps


_NC_CACHE = {}


def run_cores(in_maps, trace=False, trace_kwargs=None):
    from concourse.bass_utils import run_bass_kernel_spmd
    if "nc" not in _NC_CACHE:
        _NC_CACHE["nc"] = _build_nc()
    nc = _NC_CACHE["nc"]
    return run_bass_kernel_spmd(
        nc, in_maps, core_ids=list(range(N_CORES)),
        trace=trace, **(trace_kwargs or {}))


def kernel(hidden_states, attention_mask, position_ids, wq, wk, wv, wo):
    hidden_states = np.asarray(hidden_states, dtype=np.float32)
    position_ids = np.asarray(position_ids)
    wq = np.asarray(wq, dtype=np.float32)
    wk = np.asarray(wk, dtype=np.float32)
    wv = np.asarray(wv, dtype=np.float32)
    wo = np.asarray(wo, dtype=np.float32)

    in_maps = _host_inputs(hidden_states, position_ids, wq, wk, wv, wo)
    res = run_cores(in_maps)

    out = np.zeros((B, S, H), np.float32)
    for core in range(N_CORES):
        b = core // NKV
        out[b] += res.results[core]["oT"].T.astype(np.float32)
    return out


# revision 19
# speedup vs baseline: 1.3555x; 1.1293x over previous
"""Trainium2 Bass kernel for MimiAttention (GQA + RoPE + causal softmax).

Problem: B=2, S=2048, H=1024, NH=16 q-heads, NKV=4 kv-heads, HD=64.
Sharding: 8 cores = 2 (batch) x 4 (kv-group).  Each core computes one batch's
attention for one GQA group (4 q-heads sharing 1 kv head) and the partial
o-projection for those heads; the host sums the 4 partials per batch.

v5 design (all bf16 matmuls, fp32 psum):
  * RoPE hat-trick (as baseline): wqk columns carry [q; q2] per head, qhat =
    proj * cs; khat = [k_rot; k_rot] via the J-fold matmul.
  * Scores transposed (scoresT[j, i]) per key-tile row, streamed through two
    ping-pong [128, 1024] PSUM feed regions; ONE exp per segment (<= 2 per
    row) minimizes the ACT fixed cost.  Causal diag masked in place on Pool.
  * Software pipeline: scores+exp for row r are issued BEFORE attnV of row
    r-1 so the PE never stalls behind the Pool mask / exp chain.
  * attnV accumulates [i, v|den] slices in 3 persistent psum banks
    (65-wide slices; col 64 = denominator via the ones column of v).
  * attn[i,c] -> aT[c,i] via PE transposes through the 1-bank work slot
    (pair 0 during head 2, pair 1 inline during head 3).
  * o-projection: 2 chunks per row during head 3 through the work bank,
    remainder after attention through 3 rotating psum slots; output DMAs
    batched 4 chunks each.
"""

import numpy as np
import ml_dtypes

B, S, H = 2, 2048, 1024
NH, NKV, HD = 16, 4, 64
G = NH // NKV            # 4 q-heads per kv head
THETA = 10000.0
N_CORES = 8

BF16 = ml_dtypes.bfloat16

NSB = S // 512           # 4 chunks of 512
NST = S // 128           # 16 tiles of 128
KC = H // 128            # 8 contraction chunks
SCALE = float(1.0 / np.sqrt(HD))


def _build_nc():
    import concourse.mybir as mybir
    import concourse.tile as tile
    from concourse.tile import add_dep_helper
    from concourse import bacc

    f32 = mybir.dt.float32
    bf16 = mybir.dt.bfloat16

    nc = bacc.Bacc("TRN2", target_bir_lowering=False)

    xTd = nc.dram_tensor("xT", [H, S], bf16, kind="ExternalInput")
    wqkd = nc.dram_tensor("wqkT", [H, 640], bf16, kind="ExternalInput")
    wvd = nc.dram_tensor("wvT", [H, HD], bf16, kind="ExternalInput")
    csd = nc.dram_tensor("cs", [128, S], bf16, kind="ExternalInput")
    wod = nc.dram_tensor("woT", [G * HD, H], bf16, kind="ExternalInput")
    trid = nc.dram_tensor("trimask", [128, 128], bf16, kind="ExternalInput")
    djd = nc.dram_tensor("dupJ", [128, 128], bf16, kind="ExternalInput")
    idd = nc.dram_tensor("ident", [128, 128], bf16, kind="ExternalInput")
    oTd = nc.dram_tensor("oT", [H, S], bf16, kind="ExternalOutput")

    with tile.TileContext(nc) as tc:
        import contextlib
        ctx = contextlib.ExitStack()
        with ctx:
            consts = ctx.enter_context(tc.tile_pool(name="consts", bufs=1))
            acts = ctx.enter_context(tc.tile_pool(name="acts", bufs=1))
            ep = ctx.enter_context(tc.tile_pool(name="exps", bufs=3))
            rcp = ctx.enter_context(tc.tile_pool(name="rcp", bufs=6))
            otp = ctx.enter_context(tc.tile_pool(name="ot", bufs=3))
            pav = ctx.enter_context(
                tc.tile_pool(name="ps_av", bufs=1, space="PSUM"))
            pfa = ctx.enter_context(
                tc.tile_pool(name="ps_fa", bufs=1, space="PSUM"))
            pfb = ctx.enter_context(
                tc.tile_pool(name="ps_fb", bufs=1, space="PSUM"))
            pw = ctx.enter_context(
                tc.tile_pool(name="ps_w", bufs=1, space="PSUM"))

            # ---- input DMAs: first 512 cols of xt (per kc) + weights first
            xt_sb = consts.tile([128, KC, S], bf16, tag="xt")
            wqk_sb = consts.tile([128, KC, 640], bf16, tag="wqk")
            for kc in range(KC):
                nc.sync.dma_start(xt_sb[:, kc, 0:512],
                                  xTd[kc * 128:(kc + 1) * 128, 0:512])
                nc.sync.dma_start(
                    wqk_sb[:, kc, :], wqkd[kc * 128:(kc + 1) * 128, :])
            cs_sb = consts.tile([128, S], bf16, tag="cs")
            nc.sync.dma_start(cs_sb, csd[:, :])
            tri_sb = consts.tile([128, 128], bf16, tag="tri")
            nc.sync.dma_start(tri_sb, trid[:, :])
            dj_sb = consts.tile([128, 128], bf16, tag="dj")
            nc.sync.dma_start(dj_sb, djd[:, :])
            id_sb = consts.tile([128, 128], bf16, tag="id")
            nc.sync.dma_start(id_sb, idd[:, :])
            wv_sb = consts.tile([128, KC, HD], bf16, tag="wv")
            nc.sync.dma_start(wv_sb, wvd.rearrange("(kc p) m -> p kc m", p=128))
            for kc in range(KC):
                nc.sync.dma_start(xt_sb[:, kc, 512:S],
                                  xTd[kc * 128:(kc + 1) * 128, 512:S])
            wo_sb = consts.tile([128, 2, H], bf16, tag="wo")
            nc.sync.dma_start(wo_sb, wod.rearrange("(kc p) m -> p kc m", p=128))

            qhat = [acts.tile([128, S], bf16, tag=f"qh{m}", name=f"qhat{m}")
                    for m in range(G)]
            khat = acts.tile([128, S], bf16, tag="khat")
            ktmp = acts.tile([128, S], bf16, tag="ktmp")
            v_sb = acts.tile([128, NST, HD + 1], bf16, tag="vsb")
            attn_n = acts.tile([128, NST, G * HD], bf16, tag="attn")
            aT = acts.tile([128, 2, S], bf16, tag="aT")

            avb = [pav.tile([128, w], f32, tag=f"av{b}", name=f"avb{b}")
                   for b, w in ((0, 455), (1, 455), (2, 130))]

            def av_slice(it):
                b, o = it // 7, (it % 7) * 65
                return avb[b][:, o:o + 65]

            def feed_tile(idx, ln):
                # ping-pong exp-feed regions, allocated per segment so the
                # pool slot rotation provides the WAR chain
                if idx % 2 == 0:
                    return pfa.tile([128, ln], f32, tag="fA", name="feed",
                                    padded_shape=[128, 1024])
                return pfb.tile([128, ln], f32, tag="fB", name="feed",
                                padded_shape=[128, 1024])

            def proj_psum(m, n, ps):
                col = n * 512
                for kc in range(KC):
                    nc.tensor.matmul(
                        ps, wqk_sb[:, kc, m * 128:(m + 1) * 128],
                        xt_sb[:, kc, col:col + 512],
                        start=(kc == 0), stop=(kc == KC - 1))

            def q_chunk(h, n, ps=None):
                if ps is None:
                    ps = pw.tile([128, 512], f32, tag="w", name="psq")
                proj_psum(h, n, ps)
                col = n * 512
                nc.vector.tensor_mul(
                    qhat[h][:, col:col + 512], ps, cs_sb[:, col:col + 512])

            def k_chunk(n, ps=None, psf=None):
                if ps is None:
                    ps = pw.tile([128, 512], f32, tag="w", name="psk")
                proj_psum(G, n, ps)
                col = n * 512
                nc.vector.tensor_mul(
                    ktmp[:, col:col + 512], ps, cs_sb[:, col:col + 512])
                if psf is None:
                    psf = pw.tile([128, 512], f32, tag="w", name="psf")
                nc.tensor.matmul(psf, dj_sb, ktmp[:, col:col + 512],
                                 start=True, stop=True)
                # fold drain on ACT: it has slack outside the exp stream
                nc.scalar.copy(khat[:, col:col + 512], psf)

            def v_group(g4):
                # project 4 seq-tiles of v through one work-psum residency
                psv = pw.tile([128, 4, HD], f32, tag="w", name="psv")
                for t in range(4):
                    st = g4 * 4 + t
                    for kc in range(KC):
                        nc.tensor.matmul(
                            psv[:, t, :],
                            xt_sb[:, kc, st * 128:(st + 1) * 128],
                            wv_sb[:, kc, :],
                            start=(t == 0 and kc == 0), stop=(kc == KC - 1),
                            skip_group_check=True)
                nc.vector.tensor_copy(
                    v_sb[:, g4 * 4:g4 * 4 + 4, 0:HD], psv)

            def transpose_group(hp, g4):
                # 4 slice transposes through one work-psum residency
                psx = pw.tile([128, 4, 128], bf16, tag="w", name="pst")
                for t in range(4):
                    it = g4 * 4 + t
                    nc.tensor.matmul(
                        psx[:, t, :], attn_n[:, it, hp * 128:(hp + 1) * 128],
                        id_sb, is_transpose=True,
                        start=(t == 0), stop=True, skip_group_check=True)
                nc.vector.tensor_copy(
                    aT[:, hp, g4 * 512:(g4 + 1) * 512], psx)

            # ---- prologue
            nc.gpsimd.memset(v_sb[:, :, HD:HD + 1], 1.0)
            k_chunk(0, ps=feed_tile(0, 512), psf=pw.tile(
                [128, 512], f32, tag="w", name="psf0"))
            q_chunk(0, 0, ps=feed_tile(1, 512))
            q_chunk(0, 1, ps=pw.tile([128, 512], f32, tag="w", name="psq0"))

            seg_counter = [0]

            def scores_row(h, jt, et, mid_cb=None):
                lo = jt * 128
                cols = S - lo
                lhsT = khat[:, lo:lo + 128]
                segs = [(lo, min(1024, cols))]
                if cols > 1024:
                    segs.append((lo + 1024, cols - 1024))
                for si, (off, ln) in enumerate(segs):
                    region = feed_tile(seg_counter[0], ln)
                    seg_counter[0] += 1
                    done = 0
                    while done < ln:
                        cl = min(512, ln - done)
                        nc.tensor.matmul(
                            region[:, done:done + cl], lhsT,
                            qhat[h][:, off + done:off + done + cl],
                            start=True, stop=True)
                        done += cl
                    nc.scalar.activation(
                        et[:, off:off + ln], region[:, 0:ln],
                        mybir.ActivationFunctionType.Exp, scale=SCALE)
                    if si == 0 and mid_cb is not None:
                        mid_cb()
                # causal mask on diag tile: Pool, hidden by the pipeline
                nc.gpsimd.tensor_mul(et[:, lo:lo + 128],
                                     et[:, lo:lo + 128], tri_sb)

            attnv_state = {}   # h -> bank_first dict

            def attnv_row(h, jt, et):
                bank_first = attnv_state.setdefault(h, {})
                b1_hi = min(jt + 7, NST - 1)
                order = list(range(b1_hi, jt - 1, -1)) + \
                    list(range(NST - 1, b1_hi, -1))
                for it in order:
                    b = it // 7
                    first = jt == 0 and b not in bank_first
                    mm = nc.tensor.matmul(
                        av_slice(it), et[:, it * 128:(it + 1) * 128],
                        v_sb[:, jt, :],
                        start=first, stop=(it == jt),
                        skip_group_check=True)
                    if first:
                        bank_first[b] = mm
                    elif jt == 0:
                        add_dep_helper(mm.ins, bank_first[b].ins,
                                       sync=False,
                                       reason="bank clear first")
                pso = av_slice(jt)
                rc = rcp.tile([128, 1], f32, tag="rc", name="rc")
                nc.vector.reciprocal(rc, pso[:, HD:HD + 1])
                nc.vector.tensor_scalar_mul(
                    attn_n[:, jt, h * HD:(h + 1) * HD], pso[:, 0:HD], rc)

            # oproj -------------------------------------------------------
            oproj_pending = [(g, hc) for g in range(NSB) for hc in range(KC)]
            ot_state = {}

            def oproj_chunk(ps, drain_eng):
                g, hc = oproj_pending.pop(0)
                col = g * 512
                for kc2 in range(2):
                    nc.tensor.matmul(
                        ps, wo_sb[:, kc2, hc * 128:(hc + 1) * 128],
                        aT[:, kc2, col:col + 512],
                        start=(kc2 == 0), stop=(kc2 == 1))
                if hc % 4 == 0:
                    ot_state[g] = otp.tile([128, 4, 512], bf16, tag="otb",
                                           name="otb")
                ot = ot_state[g]
                if drain_eng == 0:
                    nc.vector.tensor_copy(ot[:, hc % 4, :], ps)
                else:
                    nc.scalar.copy(ot[:, hc % 4, :], ps)
                if hc % 4 == 3:
                    r0 = (hc // 4) * 512
                    nc.sync.dma_start(
                        oTd[r0:r0 + 512, col:col + 512].rearrange(
                            "(c p) m -> p c m", p=128), ot)

            # ---- main pipelined loop ------------------------------------
            seq = [(h, jt) for h in range(G) for jt in range(NST)]
            prev = None
            for (h, jt) in seq:
                et = ep.tile([128, S], bf16, tag="e", name=f"e{h}_{jt}")
                if h == 0 and jt == 0:
                    def row0_mid():
                        # row 0 seg_b needs qhat0 cols 1024:2048
                        q_chunk(0, 2, ps=feed_tile(1, 512))
                        q_chunk(0, 3)
                    scores_row(h, jt, et, mid_cb=row0_mid)
                    seg_counter[0] = 2   # segA used fA, segB used fB
                else:
                    scores_row(h, jt, et)
                if prev is not None:
                    attnv_row(*prev)
                prev = (h, jt, et)

                # interleaved producer work, at most ~one work-slot per row
                if h == 0:
                    if jt == 0:
                        v_group(0)
                    if jt in (2, 6, 10):
                        v_group(jt // 4 + 1)
                    if jt in (3, 7, 11):
                        k_chunk((jt + 1) // 4)
                if h < G - 1 and jt in (1, 4, 7, 10):
                    q_chunk(h + 1, (jt - 1) // 3)
                if h == 2 and jt in (3, 7, 11, 15):
                    transpose_group(0, jt // 4)
                if h == 3:
                    if jt in (5, 9, 13):
                        transpose_group(1, (jt - 5) // 4)
                    if jt >= 6 and oproj_pending and \
                            oproj_pending[0][0] * 4 + 6 <= jt:
                        ps = pw.tile([128, 512], f32, tag="w", name="psow")
                        oproj_chunk(ps, drain_eng=0)
                    if jt >= 8 and oproj_pending and \
                            oproj_pending[0][0] * 4 + 6 <= jt:
                        ps = pav.tile([128, 512], f32, tag="av0",
                                      name="psoa")
                        oproj_chunk(ps, drain_eng=0)

            # flush: last attnV row + final transposes + remaining oproj
            attnv_row(*prev)
            transpose_group(1, 3)
            ti = 0
            slots = ["w", "fA", "fB", "av0", "av1"]
            pools = {"w": pw, "fA": pfa, "fB": pfb, "av0": pav, "av1": pav}
            while oproj_pending:
                tag = slots[ti % len(slots)]
                ps = pools[tag].tile([128, 512], f32, tag=tag, name="psot")
                oproj_chunk(ps, drain_eng=ti % 2)
                ti += 1

    nc.finalize()
    return nc


def _host_inputs(hidden_states, position_ids, wq, wk, wv, wo):
    """Build the 8 per-core input maps."""
    def w2_of(w):
        # w: [64, H] rows of one head; returns sign-permuted rows
        w2 = np.empty_like(w)
        w2[:32] = -w[32:64]
        w2[32:] = w[:32]
        return w2

    trimask = np.triu(np.ones((128, 128), np.float32)).astype(BF16)
    dupJ = np.zeros((128, 128), np.float32)
    for p in range(128):
        dupJ[p, p % 64] = 1.0
        dupJ[p, p % 64 + 64] = 1.0
    dupJ = dupJ.astype(BF16)
    ident = np.eye(128, dtype=np.float32).astype(BF16)

    in_maps = []
    for core in range(N_CORES):
        b, kv = core // NKV, core % NKV
        xT = np.ascontiguousarray(hidden_states[b].T).astype(BF16)

        cols = []
        for i in range(G):
            h = kv * G + i
            wqh = wq[h * HD:(h + 1) * HD]
            cols.append(wqh.T)
            cols.append(w2_of(wqh).T)
        wkh = wk[kv * HD:(kv + 1) * HD]
        cols.append(wkh.T)
        cols.append(w2_of(wkh).T)
        wqkT = np.ascontiguousarray(np.concatenate(cols, axis=1)).astype(BF16)

        wvT = np.ascontiguousarray(wv[kv * HD:(kv + 1) * HD].T).astype(BF16)
        woT = np.ascontiguousarray(
            wo[:, kv * G * HD:(kv + 1) * G * HD].T).astype(BF16)

        inv = 1.0 / (THETA ** (np.arange(0, HD, 2, dtype=np.float32) / HD))
        freqs = position_ids[b].astype(np.float32)[:, None] * inv[None, :]
        emb = np.concatenate([freqs, freqs], axis=-1)       # [S, 64]
        cs = np.concatenate([np.cos(emb).T, np.sin(emb).T], axis=0)  # [128, S]
        cs = np.ascontiguousarray(cs).astype(BF16)

        in_maps.append({
            "xT": xT, "wqkT": wqkT, "wvT": wvT, "cs": cs, "woT": woT,
            "trimask": trimask, "dupJ": dupJ, "ident": ident,
        })
    return in_maps


_NC_CACHE = {}


def run_cores(in_maps, trace=False, trace_kwargs=None):
    from concourse.bass_utils import run_bass_kernel_spmd
    if "nc" not in _NC_CACHE:
        _NC_CACHE["nc"] = _build_nc()
    nc = _NC_CACHE["nc"]
    return run_bass_kernel_spmd(
        nc, in_maps, core_ids=list(range(N_CORES)),
        trace=trace, **(trace_kwargs or {}))


def kernel(hidden_states, attention_mask, position_ids, wq, wk, wv, wo):
    hidden_states = np.asarray(hidden_states, dtype=np.float32)
    position_ids = np.asarray(position_ids)
    wq = np.asarray(wq, dtype=np.float32)
    wk = np.asarray(wk, dtype=np.float32)
    wv = np.asarray(wv, dtype=np.float32)
    wo = np.asarray(wo, dtype=np.float32)

    in_maps = _host_inputs(hidden_states, position_ids, wq, wk, wv, wo)
    res = run_cores(in_maps)

    out = np.zeros((B, S, H), np.float32)
    for core in range(N_CORES):
        b = core // NKV
        out[b] += res.results[core]["oT"].T.astype(np.float32)
    return out


# revision 20
# speedup vs baseline: 1.4002x; 1.0330x over previous
"""Trainium2 Bass kernel for MimiAttention (GQA + RoPE + causal softmax).

Problem: B=2, S=2048, H=1024, NH=16 q-heads, NKV=4 kv-heads, HD=64.
Sharding: 8 cores = 2 (batch) x 4 (kv-group).  Each core computes one batch's
attention for one GQA group (4 q-heads sharing 1 kv head) and the partial
o-projection for those heads; the host sums the 4 partials per batch.

v5 design (all bf16 matmuls, fp32 psum):
  * RoPE hat-trick (as baseline): wqk columns carry [q; q2] per head, qhat =
    proj * cs; khat = [k_rot; k_rot] via the J-fold matmul.
  * Scores transposed (scoresT[j, i]) per key-tile row, streamed through two
    ping-pong [128, 1024] PSUM feed regions; ONE exp per segment (<= 2 per
    row) minimizes the ACT fixed cost.  Causal diag masked in place on Pool.
  * Software pipeline: scores+exp for row r are issued BEFORE attnV of row
    r-1 so the PE never stalls behind the Pool mask / exp chain.
  * attnV accumulates [i, v|den] slices in 3 persistent psum banks
    (65-wide slices; col 64 = denominator via the ones column of v).
  * attn[i,c] -> aT[c,i] via PE transposes through the 1-bank work slot
    (pair 0 during head 2, pair 1 inline during head 3).
  * o-projection: 2 chunks per row during head 3 through the work bank,
    remainder after attention through 3 rotating psum slots; output DMAs
    batched 4 chunks each.
"""

import numpy as np
import ml_dtypes

B, S, H = 2, 2048, 1024
NH, NKV, HD = 16, 4, 64
G = NH // NKV            # 4 q-heads per kv head
THETA = 10000.0
N_CORES = 8

BF16 = ml_dtypes.bfloat16

NSB = S // 512           # 4 chunks of 512
NST = S // 128           # 16 tiles of 128
KC = H // 128            # 8 contraction chunks
SCALE = float(1.0 / np.sqrt(HD))


def _build_nc():
    import concourse.mybir as mybir
    import concourse.tile as tile
    from concourse.tile import add_dep_helper
    from concourse import bacc

    f32 = mybir.dt.float32
    bf16 = mybir.dt.bfloat16

    nc = bacc.Bacc("TRN2", target_bir_lowering=False)

    xTd = nc.dram_tensor("xT", [H, S], bf16, kind="ExternalInput")
    wqkd = nc.dram_tensor("wqkT", [H, 640], bf16, kind="ExternalInput")
    wvd = nc.dram_tensor("wvT", [H, HD], bf16, kind="ExternalInput")
    csd = nc.dram_tensor("cs", [128, S], bf16, kind="ExternalInput")
    wod = nc.dram_tensor("woT", [G * HD, H], bf16, kind="ExternalInput")
    trid = nc.dram_tensor("trimask", [128, 128], bf16, kind="ExternalInput")
    djd = nc.dram_tensor("dupJ", [128, 128], bf16, kind="ExternalInput")
    idd = nc.dram_tensor("ident", [128, 128], bf16, kind="ExternalInput")
    oTd = nc.dram_tensor("oT", [H, S], bf16, kind="ExternalOutput")

    with tile.TileContext(nc) as tc:
        import contextlib
        ctx = contextlib.ExitStack()
        with ctx:
            consts = ctx.enter_context(tc.tile_pool(name="consts", bufs=1))
            acts = ctx.enter_context(tc.tile_pool(name="acts", bufs=1))
            ep = ctx.enter_context(tc.tile_pool(name="exps", bufs=3))
            rcp = ctx.enter_context(tc.tile_pool(name="rcp", bufs=6))
            otp = ctx.enter_context(tc.tile_pool(name="ot", bufs=3))
            pav = ctx.enter_context(
                tc.tile_pool(name="ps_av", bufs=1, space="PSUM"))
            pfa = ctx.enter_context(
                tc.tile_pool(name="ps_fa", bufs=1, space="PSUM"))
            pfb = ctx.enter_context(
                tc.tile_pool(name="ps_fb", bufs=1, space="PSUM"))
            pw = ctx.enter_context(
                tc.tile_pool(name="ps_w", bufs=1, space="PSUM"))

            # ---- input DMAs, ordered by first use: k weights + first xt
            # column block feed the k/q0 projections; the remaining xt lands
            # column-major so qhat chunks stream in order.
            xt_sb = consts.tile([128, KC, S], bf16, tag="xt")
            wqk_sb = consts.tile([128, KC, 640], bf16, tag="wqk")
            cs_sb = consts.tile([128, S], bf16, tag="cs")
            tri_sb = consts.tile([128, 128], bf16, tag="tri")
            dj_sb = consts.tile([128, 128], bf16, tag="dj")
            id_sb = consts.tile([128, 128], bf16, tag="id")
            wv_sb = consts.tile([128, KC, HD], bf16, tag="wv")
            wo_sb = consts.tile([128, 2, H], bf16, tag="wo")

            def xt_col(n):
                c = n * 512
                nc.sync.dma_start(
                    xt_sb[:, :, c:c + 512],
                    xTd[:, c:c + 512].rearrange("(kc p) m -> p kc m", p=128))

            def wqk_cols(c0, c1):
                nc.sync.dma_start(
                    wqk_sb[:, :, c0:c1],
                    wqkd[:, c0:c1].rearrange("(kc p) m -> p kc m", p=128))

            wqk_cols(512, 640)          # k|k2 block
            xt_col(0)
            nc.sync.dma_start(cs_sb, csd[:, :])
            nc.sync.dma_start(dj_sb, djd[:, :])
            wqk_cols(0, 128)            # q head 0
            xt_col(1)
            wqk_cols(128, 512)          # q heads 1-3
            nc.sync.dma_start(tri_sb, trid[:, :])
            xt_col(2)
            nc.sync.dma_start(wv_sb, wvd.rearrange("(kc p) m -> p kc m", p=128))
            xt_col(3)
            nc.sync.dma_start(wo_sb, wod.rearrange("(kc p) m -> p kc m", p=128))
            nc.sync.dma_start(id_sb, idd[:, :])

            qhat = [acts.tile([128, S], bf16, tag=f"qh{m}", name=f"qhat{m}")
                    for m in range(G)]
            khat = acts.tile([128, S], bf16, tag="khat")
            ktmp = acts.tile([128, S], bf16, tag="ktmp")
            v_sb = acts.tile([128, NST, HD + 1], bf16, tag="vsb")
            attn_n = acts.tile([128, NST, G * HD], bf16, tag="attn")
            aT = acts.tile([128, 2, S], bf16, tag="aT")

            avb = [pav.tile([128, w], f32, tag=f"av{b}", name=f"avb{b}")
                   for b, w in ((0, 455), (1, 455), (2, 130))]

            def av_slice(it):
                b, o = it // 7, (it % 7) * 65
                return avb[b][:, o:o + 65]

            def feed_tile(idx, ln):
                # ping-pong exp-feed regions, allocated per segment so the
                # pool slot rotation provides the WAR chain
                if idx % 2 == 0:
                    return pfa.tile([128, ln], f32, tag="fA", name="feed",
                                    padded_shape=[128, 1024])
                return pfb.tile([128, ln], f32, tag="fB", name="feed",
                                padded_shape=[128, 1024])

            def proj_psum(m, n, ps):
                col = n * 512
                for kc in range(KC):
                    nc.tensor.matmul(
                        ps, wqk_sb[:, kc, m * 128:(m + 1) * 128],
                        xt_sb[:, kc, col:col + 512],
                        start=(kc == 0), stop=(kc == KC - 1))

            def q_chunk(h, n, ps=None):
                if ps is None:
                    ps = pw.tile([128, 512], f32, tag="w", name="psq")
                proj_psum(h, n, ps)
                col = n * 512
                nc.vector.tensor_mul(
                    qhat[h][:, col:col + 512], ps, cs_sb[:, col:col + 512])

            def k_chunk(n, ps=None, psf=None):
                if ps is None:
                    ps = pw.tile([128, 512], f32, tag="w", name="psk")
                proj_psum(G, n, ps)
                col = n * 512
                nc.vector.tensor_mul(
                    ktmp[:, col:col + 512], ps, cs_sb[:, col:col + 512])
                if psf is None:
                    psf = pw.tile([128, 512], f32, tag="w", name="psf")
                nc.tensor.matmul(psf, dj_sb, ktmp[:, col:col + 512],
                                 start=True, stop=True)
                # fold drain on ACT: it has slack outside the exp stream
                nc.scalar.copy(khat[:, col:col + 512], psf)

            def v_group(g4):
                # project 4 seq-tiles of v through one work-psum residency
                psv = pw.tile([128, 4, HD], f32, tag="w", name="psv")
                for t in range(4):
                    st = g4 * 4 + t
                    for kc in range(KC):
                        nc.tensor.matmul(
                            psv[:, t, :],
                            xt_sb[:, kc, st * 128:(st + 1) * 128],
                            wv_sb[:, kc, :],
                            start=(t == 0 and kc == 0), stop=(kc == KC - 1),
                            skip_group_check=True)
                nc.vector.tensor_copy(
                    v_sb[:, g4 * 4:g4 * 4 + 4, 0:HD], psv)

            def transpose_group(hp, g4):
                # 4 slice transposes through one work-psum residency
                psx = pw.tile([128, 4, 128], bf16, tag="w", name="pst")
                for t in range(4):
                    it = g4 * 4 + t
                    nc.tensor.matmul(
                        psx[:, t, :], attn_n[:, it, hp * 128:(hp + 1) * 128],
                        id_sb, is_transpose=True,
                        start=(t == 0), stop=True, skip_group_check=True)
                nc.vector.tensor_copy(
                    aT[:, hp, g4 * 512:(g4 + 1) * 512], psx)

            # ---- prologue
            nc.gpsimd.memset(v_sb[:, :, HD:HD + 1], 1.0)
            k_chunk(0, ps=feed_tile(0, 512), psf=pw.tile(
                [128, 512], f32, tag="w", name="psf0"))
            q_chunk(0, 0, ps=feed_tile(1, 512))
            q_chunk(0, 1, ps=pw.tile([128, 512], f32, tag="w", name="psq0"))

            seg_counter = [0]

            def scores_row(h, jt, et, mid_cb=None):
                lo = jt * 128
                cols = S - lo
                lhsT = khat[:, lo:lo + 128]
                segs = [(lo, min(1024, cols))]
                if cols > 1024:
                    segs.append((lo + 1024, cols - 1024))
                for si, (off, ln) in enumerate(segs):
                    region = feed_tile(seg_counter[0], ln)
                    seg_counter[0] += 1
                    done = 0
                    while done < ln:
                        cl = min(512, ln - done)
                        nc.tensor.matmul(
                            region[:, done:done + cl], lhsT,
                            qhat[h][:, off + done:off + done + cl],
                            start=True, stop=True)
                        done += cl
                    nc.scalar.activation(
                        et[:, off:off + ln], region[:, 0:ln],
                        mybir.ActivationFunctionType.Exp, scale=SCALE)
                    if si == 0 and mid_cb is not None:
                        mid_cb()
                # causal mask on diag tile: Pool, hidden by the pipeline
                nc.gpsimd.tensor_mul(et[:, lo:lo + 128],
                                     et[:, lo:lo + 128], tri_sb)

            attnv_state = {}   # h -> bank_first dict

            def attnv_row(h, jt, et):
                bank_first = attnv_state.setdefault(h, {})
                b1_hi = min(jt + 7, NST - 1)
                order = list(range(b1_hi, jt - 1, -1)) + \
                    list(range(NST - 1, b1_hi, -1))
                for it in order:
                    b = it // 7
                    first = jt == 0 and b not in bank_first
                    mm = nc.tensor.matmul(
                        av_slice(it), et[:, it * 128:(it + 1) * 128],
                        v_sb[:, jt, :],
                        start=first, stop=(it == jt),
                        skip_group_check=True)
                    if first:
                        bank_first[b] = mm
                    elif jt == 0:
                        add_dep_helper(mm.ins, bank_first[b].ins,
                                       sync=False,
                                       reason="bank clear first")
                pso = av_slice(jt)
                rc = rcp.tile([128, 1], f32, tag="rc", name="rc")
                nc.vector.reciprocal(rc, pso[:, HD:HD + 1])
                nc.vector.tensor_scalar_mul(
                    attn_n[:, jt, h * HD:(h + 1) * HD], pso[:, 0:HD], rc)

            # oproj -------------------------------------------------------
            oproj_pending = [(g, hc) for g in range(NSB) for hc in range(KC)]
            ot_state = {}

            def oproj_chunk(ps, drain_eng):
                g, hc = oproj_pending.pop(0)
                col = g * 512
                for kc2 in range(2):
                    nc.tensor.matmul(
                        ps, wo_sb[:, kc2, hc * 128:(hc + 1) * 128],
                        aT[:, kc2, col:col + 512],
                        start=(kc2 == 0), stop=(kc2 == 1))
                if hc % 4 == 0:
                    ot_state[g] = otp.tile([128, 4, 512], bf16, tag="otb",
                                           name="otb")
                ot = ot_state[g]
                if drain_eng == 0:
                    nc.vector.tensor_copy(ot[:, hc % 4, :], ps)
                else:
                    nc.scalar.copy(ot[:, hc % 4, :], ps)
                if hc % 4 == 3:
                    r0 = (hc // 4) * 512
                    nc.sync.dma_start(
                        oTd[r0:r0 + 512, col:col + 512].rearrange(
                            "(c p) m -> p c m", p=128), ot)

            # ---- main pipelined loop ------------------------------------
            seq = [(h, jt) for h in range(G) for jt in range(NST)]
            prev = None
            for (h, jt) in seq:
                et = ep.tile([128, S], bf16, tag="e", name=f"e{h}_{jt}")
                if h == 0 and jt == 0:
                    def row0_mid():
                        # row 0 seg_b needs qhat0 cols 1024:2048
                        q_chunk(0, 2, ps=feed_tile(1, 512))
                        q_chunk(0, 3)
                    scores_row(h, jt, et, mid_cb=row0_mid)
                    seg_counter[0] = 2   # segA used fA, segB used fB
                else:
                    scores_row(h, jt, et)
                if prev is not None:
                    attnv_row(*prev)
                prev = (h, jt, et)

                # interleaved producer work, at most ~one work-slot per row
                if h == 0:
                    if jt == 0:
                        v_group(0)
                    if jt in (2, 6, 10):
                        v_group(jt // 4 + 1)
                    if jt in (3, 7, 11):
                        k_chunk((jt + 1) // 4)
                if h < G - 1 and jt in (1, 4, 7, 10):
                    q_chunk(h + 1, (jt - 1) // 3)
                if h == 2 and jt in (3, 7, 11, 15):
                    transpose_group(0, jt // 4)
                if h == 3:
                    if jt in (5, 9, 13):
                        transpose_group(1, (jt - 5) // 4)
                    if jt >= 6 and oproj_pending and \
                            oproj_pending[0][0] * 4 + 6 <= jt:
                        ps = pw.tile([128, 512], f32, tag="w", name="psow")
                        oproj_chunk(ps, drain_eng=0)
                    if jt >= 8 and oproj_pending and \
                            oproj_pending[0][0] * 4 + 6 <= jt:
                        ps = pav.tile([128, 512], f32, tag="av0",
                                      name="psoa")
                        oproj_chunk(ps, drain_eng=0)

            # flush: last attnV row + final transposes + remaining oproj
            attnv_row(*prev)
            transpose_group(1, 3)
            ti = 0
            slots = ["w", "fA", "fB", "av0", "av1"]
            pools = {"w": pw, "fA": pfa, "fB": pfb, "av0": pav, "av1": pav}
            while oproj_pending:
                tag = slots[ti % len(slots)]
                ps = pools[tag].tile([128, 512], f32, tag=tag, name="psot")
                oproj_chunk(ps, drain_eng=ti % 2)
                ti += 1

    nc.finalize()
    return nc


def _host_inputs(hidden_states, position_ids, wq, wk, wv, wo):
    """Build the 8 per-core input maps."""
    def w2_of(w):
        # w: [64, H] rows of one head; returns sign-permuted rows
        w2 = np.empty_like(w)
        w2[:32] = -w[32:64]
        w2[32:] = w[:32]
        return w2

    trimask = np.triu(np.ones((128, 128), np.float32)).astype(BF16)
    dupJ = np.zeros((128, 128), np.float32)
    for p in range(128):
        dupJ[p, p % 64] = 1.0
        dupJ[p, p % 64 + 64] = 1.0
    dupJ = dupJ.astype(BF16)
    ident = np.eye(128, dtype=np.float32).astype(BF16)

    in_maps = []
    for core in range(N_CORES):
        b, kv = core // NKV, core % NKV
        xT = np.ascontiguousarray(hidden_states[b].T).astype(BF16)

        cols = []
        for i in range(G):
            h = kv * G + i
            wqh = wq[h * HD:(h + 1) * HD]
            cols.append(wqh.T)
            cols.append(w2_of(wqh).T)
        wkh = wk[kv * HD:(kv + 1) * HD]
        cols.append(wkh.T)
        cols.append(w2_of(wkh).T)
        wqkT = np.ascontiguousarray(np.concatenate(cols, axis=1)).astype(BF16)

        wvT = np.ascontiguousarray(wv[kv * HD:(kv + 1) * HD].T).astype(BF16)
        woT = np.ascontiguousarray(
            wo[:, kv * G * HD:(kv + 1) * G * HD].T).astype(BF16)

        inv = 1.0 / (THETA ** (np.arange(0, HD, 2, dtype=np.float32) / HD))
        freqs = position_ids[b].astype(np.float32)[:, None] * inv[None, :]
        emb = np.concatenate([freqs, freqs], axis=-1)       # [S, 64]
        cs = np.concatenate([np.cos(emb).T, np.sin(emb).T], axis=0)  # [128, S]
        cs = np.ascontiguousarray(cs).astype(BF16)

        in_maps.append({
            "xT": xT, "wqkT": wqkT, "wvT": wvT, "cs": cs, "woT": woT,
            "trimask": trimask, "dupJ": dupJ, "ident": ident,
        })
    return in_maps


_NC_CACHE = {}


def run_cores(in_maps, trace=False, trace_kwargs=None):
    from concourse.bass_utils import run_bass_kernel_spmd
    if "nc" not in _NC_CACHE:
        _NC_CACHE["nc"] = _build_nc()
    nc = _NC_CACHE["nc"]
    return run_bass_kernel_spmd(
        nc, in_maps, core_ids=list(range(N_CORES)),
        trace=trace, **(trace_kwargs or {}))


def kernel(hidden_states, attention_mask, position_ids, wq, wk, wv, wo):
    hidden_states = np.asarray(hidden_states, dtype=np.float32)
    position_ids = np.asarray(position_ids)
    wq = np.asarray(wq, dtype=np.float32)
    wk = np.asarray(wk, dtype=np.float32)
    wv = np.asarray(wv, dtype=np.float32)
    wo = np.asarray(wo, dtype=np.float32)

    in_maps = _host_inputs(hidden_states, position_ids, wq, wk, wv, wo)
    res = run_cores(in_maps)

    out = np.zeros((B, S, H), np.float32)
    for core in range(N_CORES):
        b = core // NKV
        out[b] += res.results[core]["oT"].T.astype(np.float32)
    return out
